# revision 26
# baseline (speedup 1.0000x reference)
"""Trainium2 Bass kernel for CustomGRUModel.

Reference computation (per batch row):
    gx = x @ W                       # [T, 3H] input projections
    per step t:
        gh_zr = h @ U[:, :2H]
        z = sigmoid(gxz + ghz + bz)
        r = sigmoid(gxr + ghr + br)
        n = tanh(gxn + (r*h) @ U[:, 2H:] + bn)
        h = z*h + (1-z)*n
    y = h_last @ Wd + bd

Sharding: data-parallel over batch, 32 rows per core on 8 cores. Weights
replicated. No collectives.

Per-core design (v6, fp16):
  - x is pre-transposed AND pre-cast to fp16 on the host: xT [D, T, BL].
    The kernel DMAs chunk slices straight into the matmul-ready SBUF
    layout, eliminating the on-device PE transposes, PSUM staging bank
    and VectorE evict casts of v5.
  - All matmul operands fp16 (one HW pass, 2x fast-weight-load; the
    TensorE is instruction-rate-bound at ~30ns/matmul for N<=64, so
    instruction COUNT is the currency).
  - Layout "features on partitions": hT [H=512, B=32] as two persistent
    SBUF tiles h_lo/h_hi [128, 2x32].  Gate matmuls keep U tiles
    [128,128] stationary, stream hT chunks (N=32); outputs land [3H, B]
    in PSUM so elementwise runs on full 128 partitions.
  - gx = x@W is computed in chunks of TC=4 steps directly INTO the
    recurrence PSUM banks (start=True pending-zero discipline), so
    there are no per-step gx adds.  Three per-gate PSUM tiles of one
    2KB bank each, double-buffered (6 banks).
  - Distributed r: U_r@h_t = U_r@(z*h) + U_r@((1-z)*n), each part
    issued as soon as its factor exists, so the r pre-activations for
    step t+1 are complete before step t+1 begins.
  - Matmul-group ordering so PSUM m-chunks retire incrementally:
    the (1-z)n distributed-r part is ordered m-outer after the second
    zcn half so sigmoid(r) halves can start after only 4 extra MMs;
    n-gate MMs are ordered so gn m01 retires early for the first tanh
    half.  sigmoid/tanh optionally split in halves (SPLIT_ACT).
  - gx precompute units for the next chunk are spliced at the two PE
    wait-sites (before dist-r1 / before dist-r2) to keep the PE queue
    non-empty (HAM clock-gate stays at 8/8).
"""

import os

import numpy as np

B, T, D, H = 256, 512, 256, 512
NCORES = 8
BL = B // NCORES  # 32 batch rows per core
TC = 4  # timestep chunk for the gx precompute (one PSUM bank per gate)
KH = H // 128  # 4 k-tiles over H
KD = D // 128  # 2 k-tiles over D
M3H = 3 * H // 128  # 12 m-tiles over 3H

SPLIT_ACT = os.environ.get("GRU_SPLIT_ACT", "0") == "1"
# keep-warm dummy matmuls at the two PE stall sites (fills idle so the
# HAM clock-gate stays at 8/8 without delaying chain-critical matmuls)
WARM_Z = int(os.environ.get("GRU_WARM_Z", "4"))  # before n part A
WARM_A = int(os.environ.get("GRU_WARM_A", "4"))  # before dist-r1

_CACHE = {}


def _build(t_run, with_bias):
    from contextlib import ExitStack

    import concourse.bacc as bacc
    import concourse.bass as bass
    import concourse.tile as tile
    from concourse import mybir

    dt = mybir.dt
    f32 = dt.float32
    f16 = dt.float16
    AF = mybir.ActivationFunctionType

    nchunk = t_run // TC

    nc = bacc.Bacc(
        "TRN2", target_bir_lowering=False, debug=False, num_devices=NCORES
    )
    xt_d = nc.dram_tensor("xT", [D, t_run, BL], f16, kind="ExternalInput")
    w_d = nc.dram_tensor("W", [D, 3 * H], f32, kind="ExternalInput")
    u_d = nc.dram_tensor("U", [H, 3 * H], f32, kind="ExternalInput")
    b_d = nc.dram_tensor("b", [3 * H], f32, kind="ExternalInput")
    wd_d = nc.dram_tensor("Wd", [H, 1], f32, kind="ExternalInput")
    bd_d = nc.dram_tensor("bd", [1], f32, kind="ExternalInput")
    y_d = nc.dram_tensor("y", [BL, 1], f32, kind="ExternalOutput")

    # chunked view of xT: [kd, p, chunk, (t b)]
    xt_view = xt_d.rearrange("(kd p) (c t) b -> kd p c (t b)", p=128, t=TC)

    with tile.TileContext(nc) as tc, ExitStack() as ctx:
        const = ctx.enter_context(tc.tile_pool(name="const", bufs=1))
        xt_pool = ctx.enter_context(tc.tile_pool(name="xt", bufs=3))
        sb_pool = ctx.enter_context(tc.tile_pool(name="sb", bufs=3))
        gz_psum = ctx.enter_context(
            tc.tile_pool(name="gzp", bufs=2, space=bass.MemorySpace.PSUM)
        )
        gr_psum = ctx.enter_context(
            tc.tile_pool(name="grp", bufs=2, space=bass.MemorySpace.PSUM)
        )
        gn_psum = ctx.enter_context(
            tc.tile_pool(name="gnp", bufs=2, space=bass.MemorySpace.PSUM)
        )

        # ---- constants (load fp32, cast to fp16 working copies) ----
        stage = const.tile([128, 3 * H], f32)
        u_sb = const.tile([128, KH, 3 * H], f16)
        for k in range(KH):
            nc.sync.dma_start(stage[:], u_d[k * 128 : (k + 1) * 128, :])
            nc.vector.tensor_copy(u_sb[:, k, :], stage[:])
        w_sb = const.tile([128, KD, 3 * H], f16)
        for k in range(KD):
            nc.sync.dma_start(stage[:], w_d[k * 128 : (k + 1) * 128, :])
            nc.vector.tensor_copy(w_sb[:, k, :], stage[:])

        b_sb = const.tile([128, M3H], f32)
        nc.sync.dma_start(b_sb[:], b_d.rearrange("(m p) -> p m", p=128))

        wd_stage = const.tile([128, KH], f32)
        nc.sync.dma_start(wd_stage[:], wd_d.rearrange("(k p) o -> p (k o)", p=128))
        wd_sb = const.tile([128, KH], f16)
        nc.vector.tensor_copy(wd_sb[:], wd_stage[:])
        bd_sb = const.tile([1, 1], f32)
        nc.sync.dma_start(bd_sb[0:1, :], bd_d.rearrange("(o u) -> o u", u=1))
        bd_f16 = const.tile([1, 1], f16)
        nc.vector.tensor_copy(bd_f16[0:1, :], bd_sb[0:1, :])
        ones_sb = const.tile([1, BL], f16)
        nc.gpsimd.memset(ones_sb[0:1, :], 1.0)

        # persistent hidden state hT, split into halves (k-chunks 0-1 / 2-3)
        h_half = [
            const.tile([128, 2 * BL], f16, name=f"h{i}") for i in range(2)
        ]
        nc.gpsimd.memset(h_half[0][:], 0.0)
        nc.gpsimd.memset(h_half[1][:], 0.0)

        def h_slice(k):
            return h_half[k // 2][:, (k % 2) * BL : (k % 2 + 1) * BL]

        # scratch PSUM bank for keep-warm dummy matmuls (HAM stays 8/8)
        warm_pool = ctx.enter_context(
            tc.tile_pool(name="warm", bufs=1, space=bass.MemorySpace.PSUM)
        )
        warm_ps = warm_pool.tile([128, 64], f32, name="warm")

        def emit_dummies(n):
            for _ in range(n):
                nc.tensor.matmul(
                    warm_ps[:, :],
                    u_sb[:, 0, 0:128],
                    u_sb[:, 1, 0:64],
                    start=True,
                    stop=True,
                    skip_group_check=True,
                )

        gx_tiles = {}

        def make_units(c):
            """Emit-thunks for precomputing gx chunk c (TC steps) into PSUM.

            Per-gate tiles (z: m=0..3, r: 4..7, n: 8..11), layout
            [128, (t=TC)(mm=4)(b=32)] fp32 = one 2KB bank each.  The
            chronologically-first matmul into each tile (kd=0, mm=0)
            uses start=True (pending-zero: first touch overwrites,
            later touches accumulate); everything after uses
            start=False.
            """
            parts = (
                gz_psum.tile([128, TC, 4, BL], f32, name="gz", tag="gzp"),
                gr_psum.tile([128, TC, 4, BL], f32, name="gr", tag="grp"),
                gn_psum.tile([128, TC, 4, BL], f32, name="gn", tag="gnp"),
            )
            gx_tiles[c] = parts
            xt_sb = xt_pool.tile([128, KD, TC * BL], f16, name="xt", tag="xt")
            units = []

            def load(kd):
                nc.sync.dma_start(xt_sb[:, kd, :], xt_view[kd, :, c, :])

            def mm(m):
                part = parts[m // 4]
                mm_i = m % 4
                for kd in range(KD):
                    nc.tensor.matmul(
                        part[:, :, mm_i, :],
                        w_sb[:, kd, m * 128 : (m + 1) * 128],
                        xt_sb[:, kd, :],
                        start=(kd == 0 and mm_i == 0),
                        stop=False,
                        skip_group_check=True,
                    )

            for kd in range(KD):
                units.append(("dma", lambda kd=kd: load(kd)))
            for m in range(M3H):
                units.append(("mm", lambda m=m: mm(m)))
            return units

        def emit_step(c, j, site_z, site_a, site_b, site_end, dist):
            """One GRU step; gates accumulate into gx chunk tiles at t=j.

            The r-gate pre-activations for THIS step were already
            accumulated into gr during the previous step (distributed
            h-update).  This step emits the distributed r matmuls for
            the NEXT step into `dist` = (gr_tile, t, is_last) or None.

            site_a/site_b/site_end: precompute emit-thunks spliced at
            the PE wait-sites (a: before dist-r1, b: before dist-r2,
            end: after everything).
            """
            gz_t, gr_t, gn_t = gx_tiles[c]
            last = j == TC - 1

            r_sb = sb_pool.tile([128, KH * BL], f16, name="r", tag="r")
            rh_half = [
                sb_pool.tile([128, 2 * BL], f16, name=f"rh{i}", tag=f"rh{i}")
                for i in range(2)
            ]
            z_sb = sb_pool.tile([128, KH * BL], f16, name="z", tag="z")
            zc_sb = sb_pool.tile([128, KH * BL], f16, name="zc", tag="zc")
            zh_sb = sb_pool.tile([128, KH * BL], f16, name="zh", tag="zh")
            n_sb = sb_pool.tile([128, KH * BL], f16, name="n", tag="n")
            zcn_sb = sb_pool.tile([128, KH * BL], f16, name="zcn", tag="zcn")

            # --- ScalarE: sigmoid(r) (input complete since last step) ---
            if with_bias:
                for i in range(4):
                    nc.scalar.activation(
                        r_sb[:, i * BL : (i + 1) * BL],
                        gr_t[:, j, i, :],
                        AF.Sigmoid,
                        bias=b_sb[:, 4 + i : 5 + i],
                    )
            elif SPLIT_ACT:
                for i in range(2):
                    nc.scalar.activation(
                        r_sb[:, i * 2 * BL : (i + 1) * 2 * BL],
                        gr_t[:, j, 2 * i : 2 * i + 2, :].rearrange(
                            "p m b -> p (m b)"
                        ),
                        AF.Sigmoid,
                    )
            else:
                nc.scalar.activation(
                    r_sb[:],
                    gr_t[:, j, :, :].rearrange("p m b -> p (m b)"),
                    AF.Sigmoid,
                )
            # --- Vector: r*h halves ---
            for i in range(2):
                nc.vector.tensor_mul(
                    rh_half[i][:], r_sb[:, i * 2 * BL : (i + 1) * 2 * BL],
                    h_half[i][:],
                )

            # --- PE: z gates (m=0..3), h-based, k-outer ---
            for k in range(KH):
                for mm_i in range(4):
                    nc.tensor.matmul(
                        gz_t[:, j, mm_i, :],
                        u_sb[:, k, mm_i * 128 : (mm_i + 1) * 128],
                        h_slice(k),
                        start=False,
                        stop=(last and k == KH - 1 and mm_i == 3),
                        skip_group_check=True,
                    )

            # --- PE: splice site Z (fills the sigmoid(r)/rh wait) ---
            for u in site_z:
                u()
            emit_dummies(WARM_Z)

            # --- ScalarE: sigmoid(z) ---
            if with_bias:
                for i in range(4):
                    nc.scalar.activation(
                        z_sb[:, i * BL : (i + 1) * BL],
                        gz_t[:, j, i, :],
                        AF.Sigmoid,
                        bias=b_sb[:, i : i + 1],
                    )
            else:
                nc.scalar.activation(
                    z_sb[:],
                    gz_t[:, j, :, :].rearrange("p m b -> p (m b)"),
                    AF.Sigmoid,
                )
            # --- Vector: zc = 1-z ;  zh = z*h (VectorE: GpSimd is ~60%
            # slower and its lateness stalled the PE at dist-r1) ---
            nc.vector.tensor_scalar(
                zc_sb[:], z_sb[:], -1.0, 1.0,
                mybir.AluOpType.mult, mybir.AluOpType.add,
            )
            for i in range(2):
                nc.vector.tensor_mul(
                    zh_sb[:, i * 2 * BL : (i + 1) * 2 * BL],
                    z_sb[:, i * 2 * BL : (i + 1) * 2 * BL],
                    h_half[i][:],
                )

            # --- PE: n gates, part A (k=0,1 after rh half 0) ---
            for k in range(2):
                for mm_i in range(4):
                    nc.tensor.matmul(
                        gn_t[:, j, mm_i, :],
                        u_sb[:, k, (8 + mm_i) * 128 : (9 + mm_i) * 128],
                        rh_half[0][:, k * BL : (k + 1) * BL],
                        start=False,
                        stop=False,
                        skip_group_check=True,
                    )
            # part B (k=2,3 after rh half 1), m-outer so gn m-chunks
            # retire incrementally for the tanh halves
            for mm_i in range(4):
                for k in range(2, 4):
                    nc.tensor.matmul(
                        gn_t[:, j, mm_i, :],
                        u_sb[:, k, (8 + mm_i) * 128 : (9 + mm_i) * 128],
                        rh_half[1][:, (k - 2) * BL : (k - 1) * BL],
                        start=False,
                        stop=(last and mm_i == 3 and k == 3),
                        skip_group_check=True,
                    )

            # --- PE: splice site A (fills the zh wait) ---
            for u in site_a:
                u()
            emit_dummies(WARM_A)

            # --- PE: distributed r part 1 (U_r @ z*h) ---
            if dist is not None:
                ngr_t, nj, nlast = dist
                for k in range(KH):
                    for mm_i in range(4):
                        nc.tensor.matmul(
                            ngr_t[:, nj, mm_i, :],
                            u_sb[:, k, (4 + mm_i) * 128 : (5 + mm_i) * 128],
                            zh_sb[:, k * BL : (k + 1) * BL],
                            start=False,
                            stop=False,
                            skip_group_check=True,
                        )

            # --- ScalarE: tanh ---
            if with_bias:
                for i in range(4):
                    nc.scalar.activation(
                        n_sb[:, i * BL : (i + 1) * BL],
                        gn_t[:, j, i, :],
                        AF.Tanh,
                        bias=b_sb[:, 8 + i : 9 + i],
                    )
            elif SPLIT_ACT:
                for i in range(2):
                    nc.scalar.activation(
                        n_sb[:, i * 2 * BL : (i + 1) * 2 * BL],
                        gn_t[:, j, 2 * i : 2 * i + 2, :].rearrange(
                            "p m b -> p (m b)"
                        ),
                        AF.Tanh,
                    )
            else:
                nc.scalar.activation(
                    n_sb[:],
                    gn_t[:, j, :, :].rearrange("p m b -> p (m b)"),
                    AF.Tanh,
                )
            # --- Vector: zcn halves ---
            if SPLIT_ACT:
                for i in range(2):
                    nc.vector.tensor_mul(
                        zcn_sb[:, i * 2 * BL : (i + 1) * 2 * BL],
                        zc_sb[:, i * 2 * BL : (i + 1) * 2 * BL],
                        n_sb[:, i * 2 * BL : (i + 1) * 2 * BL],
                    )
            else:
                nc.vector.tensor_mul(zcn_sb[:], zc_sb[:], n_sb[:])

            # --- PE: splice site B ---
            for u in site_b:
                u()

            # --- PE: distributed r part 2 (U_r @ (1-z)*n) ---
            # zk01 after zcn half 0 (m-inner), then zk23 m-outer so gr
            # m-chunks retire incrementally for next step's sigmoid
            # halves.
            if dist is not None:
                ngr_t, nj, nlast = dist
                for mm_i in range(4):
                    for zk in range(2):
                        nc.tensor.matmul(
                            ngr_t[:, nj, mm_i, :],
                            u_sb[:, zk, (4 + mm_i) * 128 : (5 + mm_i) * 128],
                            zcn_sb[:, zk * BL : (zk + 1) * BL],
                            start=False,
                            stop=False,
                            skip_group_check=True,
                        )
                for mm_i in range(4):
                    for zk in range(2, 4):
                        nc.tensor.matmul(
                            ngr_t[:, nj, mm_i, :],
                            u_sb[:, zk, (4 + mm_i) * 128 : (5 + mm_i) * 128],
                            zcn_sb[:, zk * BL : (zk + 1) * BL],
                            start=False,
                            stop=(nlast and mm_i == 3 and zk == 3),
                            skip_group_check=True,
                        )

            # --- Vector: h = z*h + (1-z)*n ---
            for i in range(2):
                nc.vector.tensor_add(
                    h_half[i][:],
                    zh_sb[:, i * 2 * BL : (i + 1) * 2 * BL],
                    zcn_sb[:, i * 2 * BL : (i + 1) * 2 * BL],
                )

            # --- PE: remaining precompute filler ---
            for u in site_end:
                u()

        # ---- main emission ----
        for kind, u in make_units(0):
            u()
        for c in range(nchunk):
            pend = make_units(c + 1) if c + 1 < nchunk else []
            done = 0
            for j in range(TC):
                g = c * TC + j
                if g + 1 < t_run:
                    nc_, njj = divmod(g + 1, TC)
                    dist = (gx_tiles[nc_][1], njj, njj == TC - 1)
                else:
                    dist = None
                want = (len(pend) * (j + 1) + TC - 1) // TC
                batch = []
                while done < min(want, len(pend)):
                    batch.append(pend[done])
                    done += 1
                # DMA units issue at step start; mm units go to the
                # PE wait-sites (2 at A, 1 at B, rest at end).
                for kind, u in batch:
                    if kind == "dma":
                        u()
                mms = [u for kind, u in batch if kind == "mm"]
                emit_step(c, j, [], mms[:2], mms[2:3], mms[3:], dist)
            while done < len(pend):
                pend[done][1]()
                done += 1

        # final dense head: y = h @ Wd + bd
        out_ps = gz_psum.tile([128, TC, 4, BL], f32, name="outp", tag="gzp")
        for k in range(KH):
            nc.tensor.matmul(
                out_ps[0:BL, 0, 0, 0:1],
                h_slice(k),
                wd_sb[:, k : k + 1],
                start=(k == 0),
                stop=False,
            )
        nc.tensor.matmul(
            out_ps[0:BL, 0, 0, 0:1],
            ones_sb[0:1, :],
            bd_f16[0:1, :],
            start=False,
            stop=True,
        )
        y_sb = sb_pool.tile([BL, 1], f32, name="y", tag="y")
        nc.vector.tensor_copy(y_sb[:], out_ps[0:BL, 0, 0, 0:1])
        nc.sync.dma_start(y_d[:], y_sb[:])

    nc.compile()
    return nc


def kernel(x, W, U, b, Wd, bd):
    from concourse.bass_utils import run_bass_kernel_spmd

    t_run = int(os.environ.get("GRU_T_RUN", T))

    x = np.ascontiguousarray(np.asarray(x, dtype=np.float32))
    W = np.ascontiguousarray(np.asarray(W, dtype=np.float32))
    U = np.ascontiguousarray(np.asarray(U, dtype=np.float32))
    b = np.ascontiguousarray(np.asarray(b, dtype=np.float32))
    Wd = np.ascontiguousarray(np.asarray(Wd, dtype=np.float32))
    bd = np.ascontiguousarray(np.asarray(bd, dtype=np.float32))

    with_bias = bool(np.any(b != 0.0))
    key = (t_run, with_bias)
    if key not in _CACHE:
        _CACHE[key] = _build(t_run, with_bias)
    nc = _CACHE[key]

    # host-side shard + transpose + cast: per core [D, t_run, BL] fp16
    xs = x[:, :t_run, :].reshape(NCORES, BL, t_run, D).transpose(0, 3, 2, 1)
    xt = xs.astype(np.float16)  # C-contiguous copy [NCORES, D, t_run, BL]

    in_maps = [
        {
            "xT": xt[i],
            "W": W,
            "U": U,
            "b": b,
            "Wd": Wd,
            "bd": bd,
        }
        for i in range(NCORES)
    ]
    res = run_bass_kernel_spmd(
        nc,
        in_maps,
        core_ids=list(range(NCORES)),
        trace=os.environ.get("GRU_TRACE", "0") == "1",
    )
    out = np.concatenate([r["y"] for r in res.results], axis=0)
    if res.exec_time_ns is not None:
        print(f"HW exec time: {res.exec_time_ns} ns")
    return out


# revision 28
# speedup vs baseline: 1.2207x; 1.2207x over previous
"""Trainium2 Bass kernel for CustomGRUModel.

Reference computation (per batch row):
    gx = x @ W                       # [T, 3H] input projections
    per step t:
        gh_zr = h @ U[:, :2H]
        z = sigmoid(gxz + ghz + bz)
        r = sigmoid(gxr + ghr + br)
        n = tanh(gxn + (r*h) @ U[:, 2H:] + bn)
        h = z*h + (1-z)*n
    y = h_last @ Wd + bd

Sharding: data-parallel over batch, 32 rows per core on 8 cores. Weights
replicated. No collectives.

Per-core design (v6, fp16):
  - x is pre-transposed AND pre-cast to fp16 on the host: xT [D, T, BL].
    The kernel DMAs chunk slices straight into the matmul-ready SBUF
    layout, eliminating the on-device PE transposes, PSUM staging bank
    and VectorE evict casts of v5.
  - All matmul operands fp16 (one HW pass, 2x fast-weight-load; the
    TensorE is instruction-rate-bound at ~30ns/matmul for N<=64, so
    instruction COUNT is the currency).
  - Layout "features on partitions": hT [H=512, B=32] as two persistent
    SBUF tiles h_lo/h_hi [128, 2x32].  Gate matmuls keep U tiles
    [128,128] stationary, stream hT chunks (N=32); outputs land [3H, B]
    in PSUM so elementwise runs on full 128 partitions.
  - gx = x@W is computed in chunks of TC=4 steps directly INTO the
    recurrence PSUM banks (start=True pending-zero discipline), so
    there are no per-step gx adds.  Three per-gate PSUM tiles of one
    2KB bank each, double-buffered (6 banks).
  - Distributed r: U_r@h_t = U_r@(z*h) + U_r@((1-z)*n), each part
    issued as soon as its factor exists, so the r pre-activations for
    step t+1 are complete before step t+1 begins.
  - Matmul-group ordering so PSUM m-chunks retire incrementally:
    the (1-z)n distributed-r part is ordered m-outer after the second
    zcn half so sigmoid(r) halves can start after only 4 extra MMs;
    n-gate MMs are ordered so gn m01 retires early for the first tanh
    half.  sigmoid/tanh optionally split in halves (SPLIT_ACT).
  - gx precompute units for the next chunk are spliced at the two PE
    wait-sites (before dist-r1 / before dist-r2) to keep the PE queue
    non-empty (HAM clock-gate stays at 8/8).
"""

import os

import numpy as np

B, T, D, H = 256, 512, 256, 512
NCORES = 8
BL = B // NCORES  # 32 batch rows per core
TC = 4  # timestep chunk for the gx precompute (one PSUM bank per gate)
KH = H // 128  # 4 k-tiles over H
KD = D // 128  # 2 k-tiles over D
M3H = 3 * H // 128  # 12 m-tiles over 3H

SPLIT_ACT = os.environ.get("GRU_SPLIT_ACT", "0") == "1"
# keep-warm dummy matmuls at the two PE stall sites (fills idle so the
# HAM clock-gate stays at 8/8 without delaying chain-critical matmuls)
WARM_Z = int(os.environ.get("GRU_WARM_Z", "4"))  # before n part A
WARM_A = int(os.environ.get("GRU_WARM_A", "4"))  # before dist-r1

_CACHE = {}


def _build(t_run, with_bias):
    from contextlib import ExitStack

    import concourse.bacc as bacc
    import concourse.bass as bass
    import concourse.tile as tile
    from concourse import mybir

    dt = mybir.dt
    f32 = dt.float32
    f16 = dt.float16
    AF = mybir.ActivationFunctionType

    nchunk = t_run // TC

    nc = bacc.Bacc(
        "TRN2", target_bir_lowering=False, debug=False, num_devices=NCORES
    )
    xt_d = nc.dram_tensor("xT", [D, t_run, BL], f16, kind="ExternalInput")
    w_d = nc.dram_tensor("W", [D, 3 * H], f32, kind="ExternalInput")
    u_d = nc.dram_tensor("U", [H, 3 * H], f32, kind="ExternalInput")
    b_d = nc.dram_tensor("b", [3 * H], f32, kind="ExternalInput")
    wd_d = nc.dram_tensor("Wd", [H, 1], f32, kind="ExternalInput")
    bd_d = nc.dram_tensor("bd", [1], f32, kind="ExternalInput")
    y_d = nc.dram_tensor("y", [BL, 1], f32, kind="ExternalOutput")

    # chunked view of xT: [kd, p, chunk, (t b)]
    xt_view = xt_d.rearrange("(kd p) (c t) b -> kd p c (t b)", p=128, t=TC)

    with tile.TileContext(nc) as tc, ExitStack() as ctx:
        const = ctx.enter_context(tc.tile_pool(name="const", bufs=1))
        xt_pool = ctx.enter_context(tc.tile_pool(name="xt", bufs=3))
        sb_pool = ctx.enter_context(tc.tile_pool(name="sb", bufs=3))
        gz_psum = ctx.enter_context(
            tc.tile_pool(name="gzp", bufs=2, space=bass.MemorySpace.PSUM)
        )
        gr_psum = ctx.enter_context(
            tc.tile_pool(name="grp", bufs=2, space=bass.MemorySpace.PSUM)
        )
        gn_psum = ctx.enter_context(
            tc.tile_pool(name="gnp", bufs=2, space=bass.MemorySpace.PSUM)
        )

        # ---- constants (load fp32, cast to fp16 working copies) ----
        # W casts first: the opening gx chunk only needs w_sb, so the
        # PE starts ~3.5us earlier while the U casts still run.
        stage = const.tile([128, 3 * H], f32)
        w_sb = const.tile([128, KD, 3 * H], f16)
        for k in range(KD):
            nc.sync.dma_start(stage[:], w_d[k * 128 : (k + 1) * 128, :])
            nc.vector.tensor_copy(w_sb[:, k, :], stage[:])
        u_sb = const.tile([128, KH, 3 * H], f16)
        for k in range(KH):
            nc.sync.dma_start(stage[:], u_d[k * 128 : (k + 1) * 128, :])
            nc.vector.tensor_copy(u_sb[:, k, :], stage[:])

        b_sb = const.tile([128, M3H], f32)
        nc.sync.dma_start(b_sb[:], b_d.rearrange("(m p) -> p m", p=128))

        wd_stage = const.tile([128, KH], f32)
        nc.sync.dma_start(wd_stage[:], wd_d.rearrange("(k p) o -> p (k o)", p=128))
        wd_sb = const.tile([128, KH], f16)
        nc.vector.tensor_copy(wd_sb[:], wd_stage[:])
        bd_sb = const.tile([1, 1], f32)
        nc.sync.dma_start(bd_sb[0:1, :], bd_d.rearrange("(o u) -> o u", u=1))
        bd_f16 = const.tile([1, 1], f16)
        nc.vector.tensor_copy(bd_f16[0:1, :], bd_sb[0:1, :])
        ones_sb = const.tile([1, BL], f16)
        nc.gpsimd.memset(ones_sb[0:1, :], 1.0)

        # persistent hidden state hT, split into halves (k-chunks 0-1 / 2-3)
        h_half = [
            const.tile([128, 2 * BL], f16, name=f"h{i}") for i in range(2)
        ]
        nc.gpsimd.memset(h_half[0][:], 0.0)
        nc.gpsimd.memset(h_half[1][:], 0.0)

        def h_slice(k):
            return h_half[k // 2][:, (k % 2) * BL : (k % 2 + 1) * BL]

        # scratch PSUM bank for keep-warm dummy matmuls (HAM stays 8/8)
        warm_pool = ctx.enter_context(
            tc.tile_pool(name="warm", bufs=1, space=bass.MemorySpace.PSUM)
        )
        warm_ps = warm_pool.tile([128, 64], f32, name="warm")

        def emit_dummies(n):
            for _ in range(n):
                nc.tensor.matmul(
                    warm_ps[:, :],
                    u_sb[:, 0, 0:128],
                    u_sb[:, 1, 0:64],
                    start=True,
                    stop=True,
                    skip_group_check=True,
                )

        gx_tiles = {}

        def make_units(c):
            """Emit-thunks for precomputing gx chunk c (TC steps) into PSUM.

            Per-gate tiles (z: m=0..3, r: 4..7, n: 8..11), layout
            [128, (t=TC)(mm=4)(b=32)] fp32 = one 2KB bank each.  The
            chronologically-first matmul into each tile (kd=0, mm=0)
            uses start=True (pending-zero: first touch overwrites,
            later touches accumulate); everything after uses
            start=False.
            """
            parts = (
                gz_psum.tile([128, TC, 4, BL], f32, name="gz", tag="gzp"),
                gr_psum.tile([128, TC, 4, BL], f32, name="gr", tag="grp"),
                gn_psum.tile([128, TC, 4, BL], f32, name="gn", tag="gnp"),
            )
            gx_tiles[c] = parts
            xt_sb = xt_pool.tile([128, KD, TC * BL], f16, name="xt", tag="xt")
            units = []

            def load(kd):
                nc.sync.dma_start(xt_sb[:, kd, :], xt_view[kd, :, c, :])

            def mm(m):
                part = parts[m // 4]
                mm_i = m % 4
                for kd in range(KD):
                    nc.tensor.matmul(
                        part[:, :, mm_i, :],
                        w_sb[:, kd, m * 128 : (m + 1) * 128],
                        xt_sb[:, kd, :],
                        start=(kd == 0 and mm_i == 0),
                        stop=False,
                        skip_group_check=True,
                    )

            for kd in range(KD):
                units.append(("dma", lambda kd=kd: load(kd)))
            for m in range(M3H):
                units.append(("mm", lambda m=m: mm(m)))
            return units

        def emit_step(c, j, site_z, site_a, site_b, site_end, dist):
            """One GRU step; gates accumulate into gx chunk tiles at t=j.

            The r-gate pre-activations for THIS step were already
            accumulated into gr during the previous step (distributed
            h-update).  This step emits the distributed r matmuls for
            the NEXT step into `dist` = (gr_tile, t, is_last) or None.

            site_a/site_b/site_end: precompute emit-thunks spliced at
            the PE wait-sites (a: before dist-r1, b: before dist-r2,
            end: after everything).
            """
            gz_t, gr_t, gn_t = gx_tiles[c]
            last = j == TC - 1

            r_sb = sb_pool.tile([128, KH * BL], f16, name="r", tag="r")
            rh_half = [
                sb_pool.tile([128, 2 * BL], f16, name=f"rh{i}", tag=f"rh{i}")
                for i in range(2)
            ]
            z_sb = sb_pool.tile([128, KH * BL], f16, name="z", tag="z")
            zc_sb = sb_pool.tile([128, KH * BL], f16, name="zc", tag="zc")
            zh_sb = sb_pool.tile([128, KH * BL], f16, name="zh", tag="zh")
            n_sb = sb_pool.tile([128, KH * BL], f16, name="n", tag="n")
            zcn_sb = sb_pool.tile([128, KH * BL], f16, name="zcn", tag="zcn")

            # --- ScalarE: sigmoid(r) (input complete since last step) ---
            if with_bias:
                for i in range(4):
                    nc.scalar.activation(
                        r_sb[:, i * BL : (i + 1) * BL],
                        gr_t[:, j, i, :],
                        AF.Sigmoid,
                        bias=b_sb[:, 4 + i : 5 + i],
                    )
            elif SPLIT_ACT:
                for i in range(2):
                    nc.scalar.activation(
                        r_sb[:, i * 2 * BL : (i + 1) * 2 * BL],
                        gr_t[:, j, 2 * i : 2 * i + 2, :].rearrange(
                            "p m b -> p (m b)"
                        ),
                        AF.Sigmoid,
                    )
            else:
                nc.scalar.activation(
                    r_sb[:],
                    gr_t[:, j, :, :].rearrange("p m b -> p (m b)"),
                    AF.Sigmoid,
                )
            # --- Vector: r*h halves ---
            for i in range(2):
                nc.vector.tensor_mul(
                    rh_half[i][:], r_sb[:, i * 2 * BL : (i + 1) * 2 * BL],
                    h_half[i][:],
                )

            # --- PE: z gates (m=0..3), h-based, k-outer ---
            for k in range(KH):
                for mm_i in range(4):
                    nc.tensor.matmul(
                        gz_t[:, j, mm_i, :],
                        u_sb[:, k, mm_i * 128 : (mm_i + 1) * 128],
                        h_slice(k),
                        start=False,
                        stop=(last and k == KH - 1 and mm_i == 3),
                        skip_group_check=True,
                    )

            # --- PE: splice site Z (fills the sigmoid(r)/rh wait) ---
            for u in site_z:
                u()
            emit_dummies(WARM_Z)

            # --- ScalarE: sigmoid(z) ---
            if with_bias:
                for i in range(4):
                    nc.scalar.activation(
                        z_sb[:, i * BL : (i + 1) * BL],
                        gz_t[:, j, i, :],
                        AF.Sigmoid,
                        bias=b_sb[:, i : i + 1],
                    )
            else:
                nc.scalar.activation(
                    z_sb[:],
                    gz_t[:, j, :, :].rearrange("p m b -> p (m b)"),
                    AF.Sigmoid,
                )
            # --- Vector: zc = 1-z ;  zh = z*h (VectorE: GpSimd is ~60%
            # slower and its lateness stalled the PE at dist-r1) ---
            nc.vector.tensor_scalar(
                zc_sb[:], z_sb[:], -1.0, 1.0,
                mybir.AluOpType.mult, mybir.AluOpType.add,
            )
            for i in range(2):
                nc.vector.tensor_mul(
                    zh_sb[:, i * 2 * BL : (i + 1) * 2 * BL],
                    z_sb[:, i * 2 * BL : (i + 1) * 2 * BL],
                    h_half[i][:],
                )

            # --- PE: n gates, part A (k=0,1 after rh half 0) ---
            for k in range(2):
                for mm_i in range(4):
                    nc.tensor.matmul(
                        gn_t[:, j, mm_i, :],
                        u_sb[:, k, (8 + mm_i) * 128 : (9 + mm_i) * 128],
                        rh_half[0][:, k * BL : (k + 1) * BL],
                        start=False,
                        stop=False,
                        skip_group_check=True,
                    )
            # part B (k=2,3 after rh half 1), m-outer so gn m-chunks
            # retire incrementally for the tanh halves
            for mm_i in range(4):
                for k in range(2, 4):
                    nc.tensor.matmul(
                        gn_t[:, j, mm_i, :],
                        u_sb[:, k, (8 + mm_i) * 128 : (9 + mm_i) * 128],
                        rh_half[1][:, (k - 2) * BL : (k - 1) * BL],
                        start=False,
                        stop=(last and mm_i == 3 and k == 3),
                        skip_group_check=True,
                    )

            # --- PE: splice site A (fills the zh wait) ---
            for u in site_a:
                u()
            emit_dummies(WARM_A)

            # --- PE: distributed r part 1 (U_r @ z*h) ---
            if dist is not None:
                ngr_t, nj, nlast = dist
                for k in range(KH):
                    for mm_i in range(4):
                        nc.tensor.matmul(
                            ngr_t[:, nj, mm_i, :],
                            u_sb[:, k, (4 + mm_i) * 128 : (5 + mm_i) * 128],
                            zh_sb[:, k * BL : (k + 1) * BL],
                            start=False,
                            stop=False,
                            skip_group_check=True,
                        )

            # --- ScalarE: tanh ---
            if with_bias:
                for i in range(4):
                    nc.scalar.activation(
                        n_sb[:, i * BL : (i + 1) * BL],
                        gn_t[:, j, i, :],
                        AF.Tanh,
                        bias=b_sb[:, 8 + i : 9 + i],
                    )
            elif SPLIT_ACT:
                for i in range(2):
                    nc.scalar.activation(
                        n_sb[:, i * 2 * BL : (i + 1) * 2 * BL],
                        gn_t[:, j, 2 * i : 2 * i + 2, :].rearrange(
                            "p m b -> p (m b)"
                        ),
                        AF.Tanh,
                    )
            else:
                nc.scalar.activation(
                    n_sb[:],
                    gn_t[:, j, :, :].rearrange("p m b -> p (m b)"),
                    AF.Tanh,
                )
            # --- Vector: zcn halves (always split: the dist-r2 zk01
            # matmuls consume half 0 and can issue ~90ns earlier than
            # with one full-width multiply) ---
            for i in range(2):
                nc.vector.tensor_mul(
                    zcn_sb[:, i * 2 * BL : (i + 1) * 2 * BL],
                    zc_sb[:, i * 2 * BL : (i + 1) * 2 * BL],
                    n_sb[:, i * 2 * BL : (i + 1) * 2 * BL],
                )

            # --- PE: splice site B ---
            for u in site_b:
                u()

            # --- PE: distributed r part 2 (U_r @ (1-z)*n) ---
            # zk01 after zcn half 0 (m-inner), then zk23 m-outer so gr
            # m-chunks retire incrementally for next step's sigmoid
            # halves.
            if dist is not None:
                ngr_t, nj, nlast = dist
                for mm_i in range(4):
                    for zk in range(2):
                        nc.tensor.matmul(
                            ngr_t[:, nj, mm_i, :],
                            u_sb[:, zk, (4 + mm_i) * 128 : (5 + mm_i) * 128],
                            zcn_sb[:, zk * BL : (zk + 1) * BL],
                            start=False,
                            stop=False,
                            skip_group_check=True,
                        )
                for mm_i in range(4):
                    for zk in range(2, 4):
                        nc.tensor.matmul(
                            ngr_t[:, nj, mm_i, :],
                            u_sb[:, zk, (4 + mm_i) * 128 : (5 + mm_i) * 128],
                            zcn_sb[:, zk * BL : (zk + 1) * BL],
                            start=False,
                            stop=(nlast and mm_i == 3 and zk == 3),
                            skip_group_check=True,
                        )

            # --- Vector: h = z*h + (1-z)*n ---
            for i in range(2):
                nc.vector.tensor_add(
                    h_half[i][:],
                    zh_sb[:, i * 2 * BL : (i + 1) * 2 * BL],
                    zcn_sb[:, i * 2 * BL : (i + 1) * 2 * BL],
                )

            # --- PE: remaining precompute filler ---
            for u in site_end:
                u()

        # ---- main emission ----
        for kind, u in make_units(0):
            u()
        for c in range(nchunk):
            pend = make_units(c + 1) if c + 1 < nchunk else []
            done = 0
            for j in range(TC):
                g = c * TC + j
                if g + 1 < t_run:
                    nc_, njj = divmod(g + 1, TC)
                    dist = (gx_tiles[nc_][1], njj, njj == TC - 1)
                else:
                    dist = None
                want = (len(pend) * (j + 1) + TC - 1) // TC
                batch = []
                while done < min(want, len(pend)):
                    batch.append(pend[done])
                    done += 1
                # DMA units issue at step start; mm units go to the
                # PE wait-sites (2 at A, 1 at B, rest at end).
                for kind, u in batch:
                    if kind == "dma":
                        u()
                mms = [u for kind, u in batch if kind == "mm"]
                emit_step(c, j, [], mms[:2], mms[2:3], mms[3:], dist)
            while done < len(pend):
                pend[done][1]()
                done += 1

        # final dense head: y = h @ Wd + bd
        out_ps = gz_psum.tile([128, TC, 4, BL], f32, name="outp", tag="gzp")
        for k in range(KH):
            nc.tensor.matmul(
                out_ps[0:BL, 0, 0, 0:1],
                h_slice(k),
                wd_sb[:, k : k + 1],
                start=(k == 0),
                stop=False,
            )
        nc.tensor.matmul(
            out_ps[0:BL, 0, 0, 0:1],
            ones_sb[0:1, :],
            bd_f16[0:1, :],
            start=False,
            stop=True,
        )
        y_sb = sb_pool.tile([BL, 1], f32, name="y", tag="y")
        nc.vector.tensor_copy(y_sb[:], out_ps[0:BL, 0, 0, 0:1])
        nc.sync.dma_start(y_d[:], y_sb[:])

    nc.compile()
    return nc


def kernel(x, W, U, b, Wd, bd):
    from concourse.bass_utils import run_bass_kernel_spmd

    t_run = int(os.environ.get("GRU_T_RUN", T))

    x = np.ascontiguousarray(np.asarray(x, dtype=np.float32))
    W = np.ascontiguousarray(np.asarray(W, dtype=np.float32))
    U = np.ascontiguousarray(np.asarray(U, dtype=np.float32))
    b = np.ascontiguousarray(np.asarray(b, dtype=np.float32))
    Wd = np.ascontiguousarray(np.asarray(Wd, dtype=np.float32))
    bd = np.ascontiguousarray(np.asarray(bd, dtype=np.float32))

    with_bias = bool(np.any(b != 0.0))
    key = (t_run, with_bias)
    if key not in _CACHE:
        _CACHE[key] = _build(t_run, with_bias)
    nc = _CACHE[key]

    # host-side shard + transpose + cast: per core [D, t_run, BL] fp16
    xs = x[:, :t_run, :].reshape(NCORES, BL, t_run, D).transpose(0, 3, 2, 1)
    xt = xs.astype(np.float16)  # C-contiguous copy [NCORES, D, t_run, BL]

    in_maps = [
        {
            "xT": xt[i],
            "W": W,
            "U": U,
            "b": b,
            "Wd": Wd,
            "bd": bd,
        }
        for i in range(NCORES)
    ]
    res = run_bass_kernel_spmd(
        nc,
        in_maps,
        core_ids=list(range(NCORES)),
        trace=os.environ.get("GRU_TRACE", "0") == "1",
    )
    out = np.concatenate([r["y"] for r in res.results], axis=0)
    if res.exec_time_ns is not None:
        print(f"HW exec time: {res.exec_time_ns} ns")
    return out
